# revision 9
# baseline (speedup 1.0000x reference)
"""Trainium2 Bass kernel for nn_Pix_Projector2 (segment_reduce).

Computation (per batch image):
  h1 = relu(w1 @ x + b1)          x: [256, 6400]   h1: [512, 6400]
  h2 = relu(w2 @ h1 + b2)                          h2: [256, 6400]
  f2 = relu(w3 @ h2 + b3)                          f2: [128, 6400]
  f3 = bilinear_resize(f2, 80x80 -> 160x160)
  pooled[k, c] = sum_{IJ} onehot(label)[I,J,k] * f3[c,I,J] / (count[k]+eps)
  presence[k]  = count[k] > 0

Key algebraic optimization: the bilinear resize R is linear and separable
(f3 = Rh f2 Rw^T per channel), so instead of upsampling f2 (4x data) we
downsample the one-hot masks:
  A_k = Rh^T M_k Rw   (A: [8, 80, 80])
  pooled[k, c] = sum_pq A[k, pq] f2[c, pq]
  count[k] = sum_pq A[k, pq]     (rows of R sum to 1)

Sharding: batch 16 -> 2 images per core across 8 cores; weights replicated.
"""

import os
import numpy as np

import concourse.bass as bass
import concourse.bacc as bacc
import concourse.tile as tile
import concourse.mybir as mybir
from concourse import bass_utils

F32 = mybir.dt.float32
F32R = mybir.dt.float32r
BF16 = mybir.dt.bfloat16
I32 = mybir.dt.int32

N_CORES = 8
B_TOTAL = 16
B_CORE = B_TOTAL // N_CORES  # 2 images per core
CIN = 256
CMID = 512
COUT = 128
HW = 6400          # 80*80
S = 80             # low-res spatial side
S2 = 160           # label spatial side
K = 8              # classes
EPS = 1e-8
CHUNK = 400        # pq chunk (5 p-rows); 16 chunks per image
NCHUNK = HW // CHUNK
P_PER_CHUNK = CHUNK // S  # 5

AluOp = mybir.AluOpType
ActFn = mybir.ActivationFunctionType

LAST_RESULT = None  # BassKernelResults of the most recent run (for test.py)


def _resize_matrix(n_in, n_out):
    """jax.image.resize(method='bilinear', antialias=False) weight matrix
    [n_out, n_in] with half-pixel centers."""
    scale = n_out / n_in
    x = (np.arange(n_out) + 0.5) / scale - 0.5
    p = np.arange(n_in)
    w = np.maximum(0.0, 1.0 - np.abs(x[:, None] - p[None, :]))
    tot = w.sum(1, keepdims=True)
    w = np.where(tot > 0, w / tot, 0.0)
    return w.astype(np.float32)


def _build_program(with_b3: bool):
    """Builds the per-core bass program (B_CORE images per core)."""
    nc = bacc.Bacc("TRN2", target_bir_lowering=False, debug=False,
                   num_devices=N_CORES)

    # ---- DRAM I/O ----
    feat_d = nc.dram_tensor("feature", [B_CORE, CIN, HW], F32, kind="ExternalInput")
    lab_d = nc.dram_tensor("label", [B_CORE, S2, S2], I32, kind="ExternalInput")
    w1t_d = nc.dram_tensor("w1t", [CIN, CMID], F32, kind="ExternalInput")
    w2t_d = nc.dram_tensor("w2t", [CMID, CIN], F32, kind="ExternalInput")
    w3t_d = nc.dram_tensor("w3t", [CIN, COUT], BF16, kind="ExternalInput")
    b1_d = nc.dram_tensor("b1c", [128, CMID // 128], F32, kind="ExternalInput")
    b2_d = nc.dram_tensor("b2c", [128, CIN // 128], F32, kind="ExternalInput")
    rh_d = nc.dram_tensor("rh", [S2, S], BF16, kind="ExternalInput")
    if with_b3:
        b3_d = nc.dram_tensor("b3r", [1, COUT], BF16, kind="ExternalInput")
    pooled_d = nc.dram_tensor("pooled", [B_CORE, K, COUT], F32, kind="ExternalOutput")
    pres_d = nc.dram_tensor("presence", [B_CORE, K], I32, kind="ExternalOutput")

    with tile.TileContext(nc) as tc:
        import contextlib
        ctx = contextlib.ExitStack()
        with ctx:
            singles = ctx.enter_context(tc.tile_pool(name="singles", bufs=1))
            xpool = ctx.enter_context(tc.tile_pool(name="x", bufs=2 * B_CORE))
            mpool = ctx.enter_context(tc.tile_pool(name="masks", bufs=2))
            tpool = ctx.enter_context(tc.tile_pool(name="tsb", bufs=2))
            apool = ctx.enter_context(tc.tile_pool(name="asb", bufs=B_CORE))
            h1pool = ctx.enter_context(tc.tile_pool(name="h1", bufs=8))
            h2pool = ctx.enter_context(tc.tile_pool(name="h2", bufs=4))
            f2pool = ctx.enter_context(tc.tile_pool(name="f2t", bufs=3))
            xrpool = ctx.enter_context(tc.tile_pool(name="xr", bufs=4))
            smpool = ctx.enter_context(tc.tile_pool(name="small", bufs=8))
            # PSUM pools: exactly 8 banks total
            psA = ctx.enter_context(tc.tile_pool(name="psA", bufs=2, space="PSUM"))
            psB = ctx.enter_context(tc.tile_pool(name="psB", bufs=2, space="PSUM"))
            psC = ctx.enter_context(tc.tile_pool(name="psC", bufs=2, space="PSUM"))
            psP = ctx.enter_context(tc.tile_pool(name="psP", bufs=B_CORE, space="PSUM"))

            # ---- load weights / constants ----
            # fp32 weights are rounded in place to fp32r (PE requires
            # fp32r matmul operands to come from a rounding compute op)
            wstage = ctx.enter_context(tc.tile_pool(name="wstage", bufs=2))
            w1t_sb = []
            for i in range(CIN // 128):
                st = wstage.tile([128, CMID], F32, tag="wst", name=f"w1st{i}")
                nc.sync.dma_start(out=st, in_=w1t_d[i * 128:(i + 1) * 128, :])
                t = singles.tile([128, CMID], F32R, tag=f"w1t{i}")
                nc.gpsimd.tensor_copy(out=t, in_=st)
                w1t_sb.append(t)
            w2t_sb = []
            for i in range(CMID // 128):
                st = wstage.tile([128, CMID], F32, tag="wst", name=f"w2st{i}")
                nc.sync.dma_start(out=st[:, 0:CIN], in_=w2t_d[i * 128:(i + 1) * 128, :])
                t = singles.tile([128, CIN], F32R, tag=f"w2t{i}")
                nc.gpsimd.tensor_copy(out=t, in_=st[:, 0:CIN])
                w2t_sb.append(t)
            w3t_sb = []
            for i in range(CIN // 128):
                t = singles.tile([128, COUT], BF16, tag=f"w3t{i}")
                nc.sync.dma_start(out=t, in_=w3t_d[i * 128:(i + 1) * 128, :])
                w3t_sb.append(t)
            b1_sb = singles.tile([128, CMID // 128], F32, tag="b1")
            nc.sync.dma_start(out=b1_sb, in_=b1_d[:, :])
            b2_sb = singles.tile([128, CIN // 128], F32, tag="b2")
            nc.sync.dma_start(out=b2_sb, in_=b2_d[:, :])
            # resize matrix, i-chunks of (128, 32); same matrix used for rows/cols
            rh_sb1 = singles.tile([128, S], BF16, tag="rh1")
            nc.sync.dma_start(out=rh_sb1, in_=rh_d[0:128, :])
            rh_sb2 = singles.tile([32, S], BF16, tag="rh2")
            nc.sync.dma_start(out=rh_sb2, in_=rh_d[128:S2, :])
            ones_sb = singles.tile([S, 1], BF16, tag="ones")
            nc.vector.memset(ones_sb, 1.0)
            if with_b3:
                ones1_sb = singles.tile([1, S], BF16, tag="ones1")
                nc.vector.memset(ones1_sb, 1.0)
                b3_sb = singles.tile([1, COUT], BF16, tag="b3")
                nc.sync.dma_start(out=b3_sb, in_=b3_d[:, :])

            # ---- prefetch features for both images ----
            x_sb = []
            for b in range(B_CORE):
                xs = []
                for i in range(CIN // 128):
                    t = xpool.tile([128, HW], F32, tag="x")
                    nc.sync.dma_start(out=t, in_=feat_d[b, i * 128:(i + 1) * 128, :])
                    xs.append(t)
                x_sb.append(xs)

            # ---- Phase 0: masks -> A (both images) ----
            # A_sb[b]: [80(q), 8*80(k,p)] bf16 ;  A[k, p, q] at [q, k*80+p]
            a_sb = []
            for b in range(B_CORE):
                lab1 = mpool.tile([128, S2], I32, tag="lab1")
                nc.sync.dma_start(out=lab1, in_=lab_d[b, 0:128, :])
                lab2 = mpool.tile([32, S2], I32, tag="lab2")
                nc.sync.dma_start(out=lab2, in_=lab_d[b, 128:S2, :])
                # one-hot masks M[i, (k,j)]
                m1 = mpool.tile([128, K * S2], BF16, tag="m1")
                m2 = mpool.tile([32, K * S2], BF16, tag="m2")
                for k in range(K):
                    nc.vector.tensor_scalar(
                        out=m1[:, k * S2:(k + 1) * S2], in0=lab1,
                        scalar1=float(k), scalar2=None, op0=AluOp.is_equal)
                    nc.vector.tensor_scalar(
                        out=m2[:, k * S2:(k + 1) * S2], in0=lab2,
                        scalar1=float(k), scalar2=None, op0=AluOp.is_equal)
                # step 1: Tt[j, (k,p)] = sum_i M[i, j] * Rh[i, p]
                # psum tiles split so no matmul output crosses a psum bank
                tp = {}
                tp[(0, 0)] = psA.tile([128, 6 * S], F32, tag="c1", name="tp00")   # j 0:128, k 0:5
                tp[(0, 1)] = psA.tile([128, 2 * S], F32, tag="c1", name="tp01")   # j 0:128, k 6:7
                tp[(1, 0)] = psB.tile([32, 6 * S], F32, tag="c2", name="tp10")
                tp[(1, 1)] = psB.tile([32, 2 * S], F32, tag="c2", name="tp11")
                jchunks = [(0, 128), (128, 32)]
                ichunks = [(0, 128, m1, rh_sb1), (128, 32, m2, rh_sb2)]
                for jci, (j0, jn) in enumerate(jchunks):
                    for k in range(K):
                        grp = 0 if k < 6 else 1
                        col = (k if k < 6 else k - 6) * S
                        for ici, (i0, in_, mt, rht) in enumerate(ichunks):
                            nc.tensor.matmul(
                                tp[(jci, grp)][:, col:col + S],
                                lhsT=mt[:, k * S2 + j0:k * S2 + j0 + jn],
                                rhs=rht[:, :],
                                start=(ici == 0), stop=(ici == 1))
                t1s = tpool.tile([128, K * S], BF16, tag="t1s")
                t2s = tpool.tile([32, K * S], BF16, tag="t2s")
                nc.scalar.activation(out=t1s[:, 0:6 * S], in_=tp[(0, 0)], func=ActFn.Copy)
                nc.scalar.activation(out=t1s[:, 6 * S:8 * S], in_=tp[(0, 1)], func=ActFn.Copy)
                nc.scalar.activation(out=t2s[:, 0:6 * S], in_=tp[(1, 0)], func=ActFn.Copy)
                nc.scalar.activation(out=t2s[:, 6 * S:8 * S], in_=tp[(1, 1)], func=ActFn.Copy)
                # step 2: At[q, (k,p)] = sum_j Rw[j, q] * Tt[j, (k,p)]
                ap_a = psC.tile([S, 6 * S], F32, tag="f2t")
                ap_b = psC.tile([S, 2 * S], F32, tag="f2t")
                for k in range(K):
                    grp_t, col = (ap_a, k * S) if k < 6 else (ap_b, (k - 6) * S)
                    for jci, (j0, jn, ts_, rht) in enumerate(
                            [(0, 128, t1s, rh_sb1), (128, 32, t2s, rh_sb2)]):
                        nc.tensor.matmul(
                            grp_t[:, col:col + S],
                            lhsT=rht[:, :],
                            rhs=ts_[:, k * S:(k + 1) * S],
                            start=(jci == 0), stop=(jci == 1))
                at = apool.tile([S, K * S], BF16, tag="a")
                nc.scalar.activation(out=at[:, 0:6 * S], in_=ap_a, func=ActFn.Copy)
                nc.scalar.activation(out=at[:, 6 * S:8 * S], in_=ap_b, func=ActFn.Copy)
                a_sb.append(at)

            # ---- Phase 1: conv chain + pooling ----
            for b in range(B_CORE):
                a_kp = a_sb[b].rearrange("q (k p) -> q k p", k=K)
                ppool_t = psP.tile([K, COUT + 1], F32, tag="pp")
                for c in range(NCHUNK):
                    c0 = c * CHUNK
                    # round X chunk to fp32r (gpsimd; idle engine)
                    xr_t = []
                    for kk in range(CIN // 128):
                        xr = xrpool.tile([128, CHUNK], F32R, tag="xr")
                        nc.gpsimd.tensor_copy(out=xr, in_=x_sb[b][kk][:, c0:c0 + CHUNK])
                        xr_t.append(xr)
                    # conv1: h1[m*128:(m+1)*128, chunk]
                    h1_t = []
                    for m in range(CMID // 128):
                        ph = psA.tile([128, CHUNK], F32, tag="c1")
                        for kk in range(CIN // 128):
                            nc.tensor.matmul(
                                ph,
                                lhsT=w1t_sb[kk][:, m * 128:(m + 1) * 128],
                                rhs=xr_t[kk],
                                start=(kk == 0), stop=(kk == CIN // 128 - 1))
                        ht = h1pool.tile([128, CHUNK], F32R, tag="h1")
                        nc.scalar.activation(out=ht, in_=ph, func=ActFn.Relu,
                                             bias=b1_sb[:, m:m + 1])
                        h1_t.append(ht)
                    # conv2
                    h2_t = []
                    for m in range(CIN // 128):
                        ph = psB.tile([128, CHUNK], F32, tag="c2")
                        for kk in range(CMID // 128):
                            nc.tensor.matmul(
                                ph,
                                lhsT=w2t_sb[kk][:, m * 128:(m + 1) * 128],
                                rhs=h1_t[kk][:, :],
                                start=(kk == 0), stop=(kk == CMID // 128 - 1))
                        ht = h2pool.tile([128, CHUNK], BF16, tag="h2")
                        nc.scalar.activation(out=ht, in_=ph, func=ActFn.Relu,
                                             bias=b2_sb[:, m:m + 1])
                        h2_t.append(ht)
                    # conv3 (transposed output) + pooling, per p-row
                    for pp in range(P_PER_CHUNK):
                        pg = c * P_PER_CHUNK + pp
                        pf = psC.tile([S, COUT], F32, tag="f2t")
                        if with_b3:
                            nc.tensor.matmul(pf, lhsT=ones1_sb, rhs=b3_sb,
                                             start=True, stop=False)
                        for kk in range(CIN // 128):
                            nc.tensor.matmul(
                                pf,
                                lhsT=h2_t[kk][:, pp * S:(pp + 1) * S],
                                rhs=w3t_sb[kk][:, :],
                                start=(kk == 0 and not with_b3),
                                stop=(kk == CIN // 128 - 1))
                        f2t = f2pool.tile([S, COUT], BF16, tag="f2")
                        nc.vector.tensor_scalar_max(f2t, pf, 0.0)
                        # pooled[k, c] += sum_q A[k, pg, q] f2t[q, c]
                        nc.tensor.matmul(
                            ppool_t[:, 0:COUT], lhsT=a_kp[:, :, pg], rhs=f2t,
                            start=(pg == 0), stop=(pg == S - 1),
                            skip_group_check=True)
                        nc.tensor.matmul(
                            ppool_t[:, COUT:COUT + 1], lhsT=a_kp[:, :, pg],
                            rhs=ones_sb,
                            start=(pg == 0), stop=(pg == S - 1),
                            skip_group_check=True)
                # finalize image
                cnt = smpool.tile([K, 1], F32, tag="cnt")
                nc.vector.tensor_scalar_add(cnt, ppool_t[:, COUT:COUT + 1], EPS)
                rec = smpool.tile([K, 1], F32, tag="rec")
                nc.vector.reciprocal(rec, cnt)
                po = smpool.tile([K, COUT], F32, tag="po")
                nc.vector.tensor_scalar(out=po, in0=ppool_t[:, 0:COUT],
                                        scalar1=rec, scalar2=None, op0=AluOp.mult)
                pr = smpool.tile([K, 1], I32, tag="pr")
                nc.vector.tensor_scalar(out=pr, in0=ppool_t[:, COUT:COUT + 1],
                                        scalar1=0.0, scalar2=None,
                                        op0=AluOp.not_equal)
                nc.sync.dma_start(out=pooled_d[b, :, :], in_=po)
                nc.sync.dma_start(out=pres_d[b, :], in_=pr[:, 0])

    nc.compile()
    return nc


_PROGRAM_CACHE = {}


def _get_program(with_b3: bool):
    if with_b3 not in _PROGRAM_CACHE:
        _PROGRAM_CACHE[with_b3] = _build_program(with_b3)
    return _PROGRAM_CACHE[with_b3]


def kernel(feature, fusion_label, w1, b1, w2, b2, w3, b3, _trace=False):
    global LAST_RESULT
    feature = np.ascontiguousarray(np.asarray(feature, dtype=np.float32))
    label = np.ascontiguousarray(np.asarray(fusion_label).astype(np.int32))
    w1 = np.asarray(w1, dtype=np.float32)
    w2 = np.asarray(w2, dtype=np.float32)
    w3 = np.asarray(w3, dtype=np.float32)
    b1 = np.asarray(b1, dtype=np.float32)
    b2 = np.asarray(b2, dtype=np.float32)
    b3 = np.asarray(b3, dtype=np.float32)

    with_b3 = bool(np.any(b3 != 0))
    nc = _get_program(with_b3)

    bf16 = np.dtype("bfloat16") if hasattr(np, "bfloat16") else None
    import ml_dtypes
    bf16 = ml_dtypes.bfloat16

    w1t = np.ascontiguousarray(w1.T)                       # [256, 512]
    w2t = np.ascontiguousarray(w2.T)                       # [512, 256]
    w3t = np.ascontiguousarray(w3.T).astype(bf16)          # [256, 128]
    b1c = np.ascontiguousarray(b1.reshape(CMID // 128, 128).T)  # [128, 4]
    b2c = np.ascontiguousarray(b2.reshape(CIN // 128, 128).T)   # [128, 2]
    rh = _resize_matrix(S, S2).astype(bf16)                # [160, 80]

    in_maps = []
    for core in range(N_CORES):
        b0 = core * B_CORE
        m = {
            "feature": feature[b0:b0 + B_CORE].reshape(B_CORE, CIN, HW),
            "label": label[b0:b0 + B_CORE, 0],
            "w1t": w1t, "w2t": w2t, "w3t": w3t,
            "b1c": b1c, "b2c": b2c, "rh": rh,
        }
        if with_b3:
            m["b3r"] = b3.reshape(1, COUT).astype(bf16)
        in_maps.append(m)

    res = bass_utils.run_bass_kernel_spmd(
        nc, in_maps, core_ids=list(range(N_CORES)), trace=_trace)
    LAST_RESULT = res

    pooled = np.concatenate([r["pooled"] for r in res.results], axis=0)
    pres = np.concatenate([r["presence"] for r in res.results], axis=0)
    return pooled.astype(np.float32), pres.astype(np.int32)


# revision 13
# speedup vs baseline: 1.1057x; 1.1057x over previous
"""Trainium2 Bass kernel for nn_Pix_Projector2 (segment_reduce).

Computation (per batch image):
  h1 = relu(w1 @ x + b1)          x: [256, 6400]   h1: [512, 6400]
  h2 = relu(w2 @ h1 + b2)                          h2: [256, 6400]
  f2 = relu(w3 @ h2 + b3)                          f2: [128, 6400]
  f3 = bilinear_resize(f2, 80x80 -> 160x160)
  pooled[k, c] = sum_{IJ} onehot(label)[I,J,k] * f3[c,I,J] / (count[k]+eps)
  presence[k]  = count[k] > 0

Key algebraic optimization: the bilinear resize R is linear and separable
(f3 = Rh f2 Rw^T per channel), so instead of upsampling f2 (4x data) we
downsample the one-hot masks:
  A_k = Rh^T M_k Rw   (A: [8, 80, 80])
  pooled[k, c] = sum_pq A[k, pq] f2[c, pq]
  count[k] = sum_pq A[k, pq]     (rows of R sum to 1)

Sharding: batch 16 -> 2 images per core across 8 cores; weights replicated.
"""

import os
import numpy as np

import concourse.bass as bass
import concourse.bacc as bacc
import concourse.tile as tile
import concourse.mybir as mybir
from concourse import bass_utils

F32 = mybir.dt.float32
F32R = mybir.dt.float32r
BF16 = mybir.dt.bfloat16
I32 = mybir.dt.int32

N_CORES = 8
B_TOTAL = 16
B_CORE = B_TOTAL // N_CORES  # 2 images per core
CIN = 256
CMID = 512
COUT = 128
HW = 6400          # 80*80
S = 80             # low-res spatial side
S2 = 160           # label spatial side
K = 8              # classes
EPS = 1e-8
CHUNK = 400        # pq chunk (5 p-rows); 16 chunks per image
NCHUNK = HW // CHUNK
P_PER_CHUNK = CHUNK // S  # 5

AluOp = mybir.AluOpType
ActFn = mybir.ActivationFunctionType

LAST_RESULT = None  # BassKernelResults of the most recent run (for test.py)


def _resize_matrix(n_in, n_out):
    """jax.image.resize(method='bilinear', antialias=False) weight matrix
    [n_out, n_in] with half-pixel centers."""
    scale = n_out / n_in
    x = (np.arange(n_out) + 0.5) / scale - 0.5
    p = np.arange(n_in)
    w = np.maximum(0.0, 1.0 - np.abs(x[:, None] - p[None, :]))
    tot = w.sum(1, keepdims=True)
    w = np.where(tot > 0, w / tot, 0.0)
    return w.astype(np.float32)


def _build_program(with_b3: bool):
    """Builds the per-core bass program (B_CORE images per core)."""
    nc = bacc.Bacc("TRN2", target_bir_lowering=False, debug=False,
                   num_devices=N_CORES)

    # ---- DRAM I/O ----
    feat_d = nc.dram_tensor("feature", [B_CORE, CIN, HW], F32, kind="ExternalInput")
    lab_d = nc.dram_tensor("label", [B_CORE, S2, S2], I32, kind="ExternalInput")
    w1t_d = nc.dram_tensor("w1t", [CIN, CMID], F32, kind="ExternalInput")
    w2t_d = nc.dram_tensor("w2t", [CMID, CIN], F32, kind="ExternalInput")
    w3t_d = nc.dram_tensor("w3t", [CIN, COUT], BF16, kind="ExternalInput")
    b1_d = nc.dram_tensor("b1c", [128, CMID // 128], F32, kind="ExternalInput")
    b2_d = nc.dram_tensor("b2c", [128, CIN // 128], F32, kind="ExternalInput")
    rh_d = nc.dram_tensor("rh", [S2, S], BF16, kind="ExternalInput")
    if with_b3:
        b3_d = nc.dram_tensor("b3r", [1, COUT], BF16, kind="ExternalInput")
    pooled_d = nc.dram_tensor("pooled", [B_CORE, K, COUT], F32, kind="ExternalOutput")
    pres_d = nc.dram_tensor("presence", [B_CORE, K], I32, kind="ExternalOutput")
    DEBUG = bool(int(os.environ.get("KERNEL_DEBUG", "0")))
    if DEBUG:
        dbg_cnt_d = nc.dram_tensor("dbg_cnt", [B_CORE, K], F32, kind="ExternalOutput")
        dbg_f2_d = nc.dram_tensor("dbg_f2", [B_CORE, S, S, COUT], F32, kind="ExternalOutput")
        dbg_h1_d = nc.dram_tensor("dbg_h1", [B_CORE, 128, CHUNK], F32, kind="ExternalOutput")
        dbg_h2_d = nc.dram_tensor("dbg_h2", [B_CORE, 128, CHUNK], F32, kind="ExternalOutput")
        dbg_a_d = nc.dram_tensor("dbg_a", [B_CORE, S, K * S], F32, kind="ExternalOutput")

    with tile.TileContext(nc) as tc:
        import contextlib
        ctx = contextlib.ExitStack()
        with ctx:
            singles = ctx.enter_context(tc.tile_pool(name="singles", bufs=1))
            xpool = ctx.enter_context(tc.tile_pool(name="x", bufs=2 * B_CORE))
            mpool = ctx.enter_context(tc.tile_pool(name="masks", bufs=2))
            tpool = ctx.enter_context(tc.tile_pool(name="tsb", bufs=2))
            apool = ctx.enter_context(tc.tile_pool(name="asb", bufs=B_CORE))
            h1pool = ctx.enter_context(tc.tile_pool(name="h1", bufs=8))
            h2pool = ctx.enter_context(tc.tile_pool(name="h2", bufs=4))
            f2pool = ctx.enter_context(tc.tile_pool(name="f2t", bufs=3))
            xrpool = ctx.enter_context(tc.tile_pool(name="xr", bufs=4))
            smpool = ctx.enter_context(tc.tile_pool(name="small", bufs=8))
            # PSUM pools: exactly 8 banks total
            psA = ctx.enter_context(tc.tile_pool(name="psA", bufs=2, space="PSUM"))
            psB = ctx.enter_context(tc.tile_pool(name="psB", bufs=2, space="PSUM"))
            psC = ctx.enter_context(tc.tile_pool(name="psC", bufs=2, space="PSUM"))
            psP = ctx.enter_context(tc.tile_pool(name="psP", bufs=B_CORE, space="PSUM"))

            # ---- load weights / constants ----
            # fp32 weights are rounded in place to fp32r (PE requires
            # fp32r matmul operands to come from a rounding compute op)
            wstage = ctx.enter_context(tc.tile_pool(name="wstage", bufs=2))
            w1t_sb = []
            for i in range(CIN // 128):
                st = wstage.tile([128, CMID], F32, tag="wst", name=f"w1st{i}")
                nc.sync.dma_start(out=st, in_=w1t_d[i * 128:(i + 1) * 128, :])
                t = singles.tile([128, CMID], F32R, tag=f"w1t{i}")
                nc.gpsimd.tensor_copy(out=t, in_=st)
                w1t_sb.append(t)
            w2t_sb = []
            for i in range(CMID // 128):
                st = wstage.tile([128, CMID], F32, tag="wst", name=f"w2st{i}")
                nc.sync.dma_start(out=st[:, 0:CIN], in_=w2t_d[i * 128:(i + 1) * 128, :])
                t = singles.tile([128, CIN], F32R, tag=f"w2t{i}")
                nc.gpsimd.tensor_copy(out=t, in_=st[:, 0:CIN])
                w2t_sb.append(t)
            w3t_sb = []
            for i in range(CIN // 128):
                t = singles.tile([128, COUT], BF16, tag=f"w3t{i}")
                nc.sync.dma_start(out=t, in_=w3t_d[i * 128:(i + 1) * 128, :])
                w3t_sb.append(t)
            b1_sb = singles.tile([128, CMID // 128], F32, tag="b1")
            nc.sync.dma_start(out=b1_sb, in_=b1_d[:, :])
            b2_sb = singles.tile([128, CIN // 128], F32, tag="b2")
            nc.sync.dma_start(out=b2_sb, in_=b2_d[:, :])
            # resize matrix, i-chunks of (128, 32); same matrix used for rows/cols
            rh_sb1 = singles.tile([128, S], BF16, tag="rh1")
            nc.sync.dma_start(out=rh_sb1, in_=rh_d[0:128, :])
            rh_sb2 = singles.tile([32, S], BF16, tag="rh2")
            nc.sync.dma_start(out=rh_sb2, in_=rh_d[128:S2, :])
            # f2t tiles are rotated manually so their ones-column (used to
            # accumulate per-class counts in the pooling matmul) is set once
            f2t_tiles = []
            for i in range(3):
                ft = f2pool.tile([S, COUT + 1], BF16, tag="f2", name=f"f2t{i}")
                nc.vector.memset(ft[:, COUT:COUT + 1], 1.0)
                f2t_tiles.append(ft)
            if with_b3:
                ones1_sb = singles.tile([1, S], BF16, tag="ones1")
                nc.vector.memset(ones1_sb, 1.0)
                b3_sb = singles.tile([1, COUT], BF16, tag="b3")
                nc.sync.dma_start(out=b3_sb, in_=b3_d[:, :])

            # ---- prefetch features for both images ----
            x_sb = []
            for b in range(B_CORE):
                xs = []
                for i in range(CIN // 128):
                    t = xpool.tile([128, HW], F32, tag="x")
                    nc.sync.dma_start(out=t, in_=feat_d[b, i * 128:(i + 1) * 128, :])
                    xs.append(t)
                x_sb.append(xs)

            # ---- Phase 0: masks -> A (both images) ----
            # A_sb[b]: [80(q), 8*80(k,p)] bf16 ;  A[k, p, q] at [q, k*80+p]
            a_sb = []
            for b in range(B_CORE):
                lab1 = mpool.tile([128, S2], I32, tag="lab1")
                nc.sync.dma_start(out=lab1, in_=lab_d[b, 0:128, :])
                lab2 = mpool.tile([32, S2], I32, tag="lab2")
                nc.sync.dma_start(out=lab2, in_=lab_d[b, 128:S2, :])
                # one-hot masks M[i, (k,j)]
                m1 = mpool.tile([128, K * S2], BF16, tag="m1")
                m2 = mpool.tile([32, K * S2], BF16, tag="m2")
                for k in range(K):
                    nc.vector.tensor_scalar(
                        out=m1[:, k * S2:(k + 1) * S2], in0=lab1,
                        scalar1=float(k), scalar2=None, op0=AluOp.is_equal)
                    nc.vector.tensor_scalar(
                        out=m2[:, k * S2:(k + 1) * S2], in0=lab2,
                        scalar1=float(k), scalar2=None, op0=AluOp.is_equal)
                # step 1: Tt[j, (k,p)] = sum_i M[i, j] * Rh[i, p]
                # psum tiles split so no matmul output crosses a psum bank
                tp = {}
                tp[(0, 0)] = psA.tile([128, 6 * S], F32, tag="c1", name="tp00")   # j 0:128, k 0:5
                tp[(0, 1)] = psA.tile([128, 2 * S], F32, tag="c1", name="tp01")   # j 0:128, k 6:7
                tp[(1, 0)] = psB.tile([32, 6 * S], F32, tag="c2", name="tp10")
                tp[(1, 1)] = psB.tile([32, 2 * S], F32, tag="c2", name="tp11")
                jchunks = [(0, 128), (128, 32)]
                ichunks = [(0, 128, m1, rh_sb1), (128, 32, m2, rh_sb2)]
                for jci, (j0, jn) in enumerate(jchunks):
                    for k in range(K):
                        grp = 0 if k < 6 else 1
                        col = (k if k < 6 else k - 6) * S
                        for ici, (i0, in_, mt, rht) in enumerate(ichunks):
                            nc.tensor.matmul(
                                tp[(jci, grp)][:, col:col + S],
                                lhsT=mt[:, k * S2 + j0:k * S2 + j0 + jn],
                                rhs=rht[:, :],
                                start=(ici == 0), stop=(ici == 1))
                t1s = tpool.tile([128, K * S], BF16, tag="t1s")
                t2s = tpool.tile([32, K * S], BF16, tag="t2s")
                nc.scalar.activation(out=t1s[:, 0:6 * S], in_=tp[(0, 0)], func=ActFn.Copy)
                nc.scalar.activation(out=t1s[:, 6 * S:8 * S], in_=tp[(0, 1)], func=ActFn.Copy)
                nc.scalar.activation(out=t2s[:, 0:6 * S], in_=tp[(1, 0)], func=ActFn.Copy)
                nc.scalar.activation(out=t2s[:, 6 * S:8 * S], in_=tp[(1, 1)], func=ActFn.Copy)
                # step 2: At[q, (k,p)] = sum_j Rw[j, q] * Tt[j, (k,p)]
                ap_a = psC.tile([S, 6 * S], F32, tag="f2t")
                ap_b = psC.tile([S, 2 * S], F32, tag="f2t")
                for k in range(K):
                    grp_t, col = (ap_a, k * S) if k < 6 else (ap_b, (k - 6) * S)
                    for jci, (j0, jn, ts_, rht) in enumerate(
                            [(0, 128, t1s, rh_sb1), (128, 32, t2s, rh_sb2)]):
                        nc.tensor.matmul(
                            grp_t[:, col:col + S],
                            lhsT=rht[:, :],
                            rhs=ts_[:, k * S:(k + 1) * S],
                            start=(jci == 0), stop=(jci == 1))
                at = apool.tile([S, K * S], BF16, tag="a")
                nc.scalar.activation(out=at[:, 0:6 * S], in_=ap_a, func=ActFn.Copy)
                nc.scalar.activation(out=at[:, 6 * S:8 * S], in_=ap_b, func=ActFn.Copy)
                a_sb.append(at)
                if DEBUG:
                    ad = smpool.tile([S, K * S], F32, tag="ad")
                    nc.vector.tensor_copy(out=ad, in_=at)
                    nc.sync.dma_start(out=dbg_a_d[b], in_=ad)

            # ---- Phase 1: conv chain + pooling ----
            for b in range(B_CORE):
                a_kp = a_sb[b].rearrange("q (k p) -> q k p", k=K)
                ppool_t = psP.tile([K, COUT + 1], F32, tag="pp")
                for c in range(NCHUNK):
                    c0 = c * CHUNK
                    # round X chunk to fp32r (gpsimd; idle engine)
                    xr_t = []
                    for kk in range(CIN // 128):
                        xr = xrpool.tile([128, CHUNK], F32R, tag="xr")
                        nc.gpsimd.tensor_copy(out=xr, in_=x_sb[b][kk][:, c0:c0 + CHUNK])
                        xr_t.append(xr)
                    # conv1: h1[m*128:(m+1)*128, chunk]
                    h1_t = []
                    for m in range(CMID // 128):
                        ph = psA.tile([128, CHUNK], F32, tag="c1")
                        for kk in range(CIN // 128):
                            nc.tensor.matmul(
                                ph,
                                lhsT=w1t_sb[kk][:, m * 128:(m + 1) * 128],
                                rhs=xr_t[kk],
                                start=(kk == 0), stop=(kk == CIN // 128 - 1))
                        ht = h1pool.tile([128, CHUNK], F32R, tag="h1")
                        nc.scalar.activation(out=ht, in_=ph, func=ActFn.Relu,
                                             bias=b1_sb[:, m:m + 1])
                        h1_t.append(ht)
                        if DEBUG and c == 0 and m == 0:
                            nc.sync.dma_start(out=dbg_h1_d[b], in_=ht.bitcast(F32))
                    # conv2
                    h2_t = []
                    for m in range(CIN // 128):
                        ph = psB.tile([128, CHUNK], F32, tag="c2")
                        for kk in range(CMID // 128):
                            nc.tensor.matmul(
                                ph,
                                lhsT=w2t_sb[kk][:, m * 128:(m + 1) * 128],
                                rhs=h1_t[kk][:, :],
                                start=(kk == 0), stop=(kk == CMID // 128 - 1))
                        ht = h2pool.tile([128, CHUNK], BF16, tag="h2")
                        nc.scalar.activation(out=ht, in_=ph, func=ActFn.Relu,
                                             bias=b2_sb[:, m:m + 1])
                        h2_t.append(ht)
                        if DEBUG and c == 0 and m == 0:
                            h2d = smpool.tile([128, CHUNK], F32, tag="h2d")
                            nc.vector.tensor_copy(out=h2d, in_=ht)
                            nc.sync.dma_start(out=dbg_h2_d[b], in_=h2d)
                    # conv3 (transposed output) + pooling, per p-row
                    for pp in range(P_PER_CHUNK):
                        pg = c * P_PER_CHUNK + pp
                        pf = psC.tile([S, COUT], F32, tag="f2t")
                        if with_b3:
                            nc.tensor.matmul(pf, lhsT=ones1_sb, rhs=b3_sb,
                                             start=True, stop=False)
                        for kk in range(CIN // 128):
                            nc.tensor.matmul(
                                pf,
                                lhsT=h2_t[kk][:, pp * S:(pp + 1) * S],
                                rhs=w3t_sb[kk][:, :],
                                start=(kk == 0 and not with_b3),
                                stop=(kk == CIN // 128 - 1))
                        f2t = f2t_tiles[pg % 3]
                        nc.vector.tensor_scalar_max(f2t[:, 0:COUT], pf, 0.0)
                        if DEBUG:
                            f2d = smpool.tile([S, COUT], F32, tag="f2d")
                            nc.vector.tensor_copy(out=f2d, in_=f2t[:, 0:COUT])
                            nc.sync.dma_start(out=dbg_f2_d[b, pg], in_=f2d)
                        # pooled[k, :COUT] += sum_q A[k, pg, q] f2t[q, :]
                        # last (ones) column accumulates the per-class counts
                        nc.tensor.matmul(
                            ppool_t, lhsT=a_kp[:, :, pg], rhs=f2t,
                            start=(pg == 0), stop=(pg == S - 1),
                            skip_group_check=True)
                # finalize image
                cnt = smpool.tile([K, 1], F32, tag="cnt")
                nc.vector.tensor_scalar_add(cnt, ppool_t[:, COUT:COUT + 1], EPS)
                rec = smpool.tile([K, 1], F32, tag="rec")
                nc.vector.reciprocal(rec, cnt)
                po = smpool.tile([K, COUT], F32, tag="po")
                nc.vector.tensor_scalar(out=po, in0=ppool_t[:, 0:COUT],
                                        scalar1=rec, scalar2=None, op0=AluOp.mult)
                if DEBUG:
                    cntd = smpool.tile([K, 1], F32, tag="cntd")
                    nc.vector.tensor_copy(out=cntd, in_=ppool_t[:, COUT:COUT + 1])
                    nc.sync.dma_start(out=dbg_cnt_d[b, :], in_=cntd[:, 0:1])
                pr = smpool.tile([K, 1], I32, tag="pr")
                nc.vector.tensor_scalar(out=pr, in0=ppool_t[:, COUT:COUT + 1],
                                        scalar1=0.0, scalar2=None,
                                        op0=AluOp.not_equal)
                nc.sync.dma_start(out=pooled_d[b, :, :], in_=po)
                nc.sync.dma_start(out=pres_d[b, :], in_=pr[:, 0])

    nc.compile()
    return nc


_PROGRAM_CACHE = {}


def _get_program(with_b3: bool):
    if with_b3 not in _PROGRAM_CACHE:
        _PROGRAM_CACHE[with_b3] = _build_program(with_b3)
    return _PROGRAM_CACHE[with_b3]


def kernel(feature, fusion_label, w1, b1, w2, b2, w3, b3, _trace=False):
    global LAST_RESULT
    feature = np.ascontiguousarray(np.asarray(feature, dtype=np.float32))
    label = np.ascontiguousarray(np.asarray(fusion_label).astype(np.int32))
    w1 = np.asarray(w1, dtype=np.float32)
    w2 = np.asarray(w2, dtype=np.float32)
    w3 = np.asarray(w3, dtype=np.float32)
    b1 = np.asarray(b1, dtype=np.float32)
    b2 = np.asarray(b2, dtype=np.float32)
    b3 = np.asarray(b3, dtype=np.float32)

    with_b3 = bool(np.any(b3 != 0))
    nc = _get_program(with_b3)

    bf16 = np.dtype("bfloat16") if hasattr(np, "bfloat16") else None
    import ml_dtypes
    bf16 = ml_dtypes.bfloat16

    w1t = np.ascontiguousarray(w1.T)                       # [256, 512]
    w2t = np.ascontiguousarray(w2.T)                       # [512, 256]
    w3t = np.ascontiguousarray(w3.T).astype(bf16)          # [256, 128]
    b1c = np.ascontiguousarray(b1.reshape(CMID // 128, 128).T)  # [128, 4]
    b2c = np.ascontiguousarray(b2.reshape(CIN // 128, 128).T)   # [128, 2]
    rh = _resize_matrix(S, S2).astype(bf16)                # [160, 80]

    in_maps = []
    for core in range(N_CORES):
        b0 = core * B_CORE
        m = {
            "feature": feature[b0:b0 + B_CORE].reshape(B_CORE, CIN, HW),
            "label": label[b0:b0 + B_CORE, 0],
            "w1t": w1t, "w2t": w2t, "w3t": w3t,
            "b1c": b1c, "b2c": b2c, "rh": rh,
        }
        if with_b3:
            m["b3r"] = b3.reshape(1, COUT).astype(bf16)
        in_maps.append(m)

    res = bass_utils.run_bass_kernel_spmd(
        nc, in_maps, core_ids=list(range(N_CORES)), trace=_trace)
    LAST_RESULT = res

    pooled = np.concatenate([r["pooled"] for r in res.results], axis=0)
    pres = np.concatenate([r["presence"] for r in res.results], axis=0)
    return pooled.astype(np.float32), pres.astype(np.int32)


# revision 19
# speedup vs baseline: 1.1166x; 1.0099x over previous
"""Trainium2 Bass kernel for nn_Pix_Projector2 (segment_reduce).

Computation (per batch image):
  h1 = relu(w1 @ x + b1)          x: [256, 6400]   h1: [512, 6400]
  h2 = relu(w2 @ h1 + b2)                          h2: [256, 6400]
  f2 = relu(w3 @ h2 + b3)                          f2: [128, 6400]
  f3 = bilinear_resize(f2, 80x80 -> 160x160)
  pooled[k, c] = sum_{IJ} onehot(label)[I,J,k] * f3[c,I,J] / (count[k]+eps)
  presence[k]  = count[k] > 0

Key algebraic optimization: the bilinear resize R is linear and separable
(f3 = Rh f2 Rw^T per channel), so instead of upsampling f2 (4x data) we
downsample the one-hot masks:
  A_k = Rh^T M_k Rw   (A: [8, 80, 80], exactly representable in bf16)
  pooled[k, c] = sum_pq A[k, pq] f2[c, pq]
  count[k] = sum_pq A[k, pq]     (rows of R sum to 1 -> counts exact)

conv3 uses H2 chunks as the stationary operand so its output lands
transposed ([pq, c]), which the pooling matmul needs (contraction over pq
must be on the partition axis). Pooling accumulates [8, 129] (128
channels + a ones-column that yields the per-class counts) over 80 p-row
matmuls in a single PSUM accumulation group.

Sharding: batch 16 -> 2 images per core across 8 cores; weights replicated.
"""

import os
import numpy as np

import concourse.bass as bass
import concourse.bacc as bacc
import concourse.tile as tile
import concourse.mybir as mybir
from concourse import bass_utils

F32 = mybir.dt.float32
BF16 = mybir.dt.bfloat16
I32 = mybir.dt.int32

N_CORES = 8
B_TOTAL = 16
B_CORE = B_TOTAL // N_CORES  # 2 images per core
CIN = 256
CMID = 512
COUT = 128
HW = 6400          # 80*80
S = 80             # low-res spatial side
S2 = 160           # label spatial side
K = 8              # classes
EPS = 1e-8
CHUNKS = [512] * 12 + [256]   # pq chunking for conv1/conv2

AluOp = mybir.AluOpType
ActFn = mybir.ActivationFunctionType

LAST_RESULT = None  # BassKernelResults of the most recent run (for test.py)


def _resize_matrix(n_in, n_out):
    """jax.image.resize(method='bilinear', antialias=False) weight matrix
    [n_out, n_in] with half-pixel centers."""
    scale = n_out / n_in
    x = (np.arange(n_out) + 0.5) / scale - 0.5
    p = np.arange(n_in)
    w = np.maximum(0.0, 1.0 - np.abs(x[:, None] - p[None, :]))
    tot = w.sum(1, keepdims=True)
    w = np.where(tot > 0, w / tot, 0.0)
    return w.astype(np.float32)


def _build_program(with_b3: bool, reps: int = 1, skip=()):
    """Builds the per-core bass program. `reps`/`skip` are for timing only."""
    nc = bacc.Bacc("TRN2", target_bir_lowering=False, debug=False,
                   num_devices=N_CORES)

    feat_d = nc.dram_tensor("feature", [B_CORE, CIN, HW], F32, kind="ExternalInput")
    lab_d = nc.dram_tensor("label", [B_CORE, S2, S2], I32, kind="ExternalInput")
    w1t_d = nc.dram_tensor("w1t", [CIN, CMID], BF16, kind="ExternalInput")
    w2t_d = nc.dram_tensor("w2t", [CMID, CIN], BF16, kind="ExternalInput")
    w3t_d = nc.dram_tensor("w3t", [CIN, COUT], BF16, kind="ExternalInput")
    b1_d = nc.dram_tensor("b1c", [128, CMID // 128], F32, kind="ExternalInput")
    b2_d = nc.dram_tensor("b2c", [128, CIN // 128], F32, kind="ExternalInput")
    rh_d = nc.dram_tensor("rh", [S2, S], BF16, kind="ExternalInput")
    if with_b3:
        b3_d = nc.dram_tensor("b3r", [1, COUT], BF16, kind="ExternalInput")
    pooled_d = nc.dram_tensor("pooled", [B_CORE, K, COUT], F32, kind="ExternalOutput")
    pres_d = nc.dram_tensor("presence", [B_CORE, K], I32, kind="ExternalOutput")

    with tile.TileContext(nc) as tc:
        import contextlib
        ctx = contextlib.ExitStack()
        with ctx:
            singles = ctx.enter_context(tc.tile_pool(name="singles", bufs=1))
            xpool = ctx.enter_context(tc.tile_pool(name="x", bufs=2 * B_CORE))
            mpool = ctx.enter_context(tc.tile_pool(name="masks", bufs=2))
            tpool = ctx.enter_context(tc.tile_pool(name="tsb", bufs=2))
            apool = ctx.enter_context(tc.tile_pool(name="asb", bufs=B_CORE))
            h1pool = ctx.enter_context(tc.tile_pool(name="h1", bufs=8))
            h2pool = ctx.enter_context(tc.tile_pool(name="h2big", bufs=2 * B_CORE))
            f2pool = ctx.enter_context(tc.tile_pool(name="f2t", bufs=1))
            smpool = ctx.enter_context(tc.tile_pool(name="small", bufs=4))
            # PSUM: psA 3 + psB 2 + psC 2 + psP 1 = 8 banks
            psA = ctx.enter_context(tc.tile_pool(name="psA", bufs=3, space="PSUM"))
            psB = ctx.enter_context(tc.tile_pool(name="psB", bufs=2, space="PSUM"))
            psC = ctx.enter_context(tc.tile_pool(name="psC", bufs=2, space="PSUM"))
            psP = ctx.enter_context(tc.tile_pool(name="psP", bufs=1, space="PSUM"))

            # ---- weights / constants ----
            w1t_sb = []
            for i in range(CIN // 128):
                t = singles.tile([128, CMID], BF16, tag=f"w1t{i}")
                nc.sync.dma_start(out=t, in_=w1t_d[i * 128:(i + 1) * 128, :])
                w1t_sb.append(t)
            w2t_sb = []
            for i in range(CMID // 128):
                t = singles.tile([128, CIN], BF16, tag=f"w2t{i}")
                nc.sync.dma_start(out=t, in_=w2t_d[i * 128:(i + 1) * 128, :])
                w2t_sb.append(t)
            w3t_sb = []
            for i in range(CIN // 128):
                t = singles.tile([128, COUT], BF16, tag=f"w3t{i}")
                nc.sync.dma_start(out=t, in_=w3t_d[i * 128:(i + 1) * 128, :])
                w3t_sb.append(t)
            b1_sb = singles.tile([128, CMID // 128], F32, tag="b1")
            nc.sync.dma_start(out=b1_sb, in_=b1_d[:, :])
            b2_sb = singles.tile([128, CIN // 128], F32, tag="b2")
            nc.sync.dma_start(out=b2_sb, in_=b2_d[:, :])
            rh_sb1 = singles.tile([128, S], BF16, tag="rh1")
            nc.sync.dma_start(out=rh_sb1, in_=rh_d[0:128, :])
            rh_sb2 = singles.tile([32, S], BF16, tag="rh2")
            nc.sync.dma_start(out=rh_sb2, in_=rh_d[128:S2, :])
            # f2t tiles rotated manually; ones-column (count accumulator) set once
            f2t_tiles = []
            for i in range(3):
                ft = f2pool.tile([S, COUT + 1], BF16, tag=f"f2_{i}", name=f"f2t{i}")
                nc.vector.memset(ft[:, COUT:COUT + 1], 1.0)
                f2t_tiles.append(ft)
            if with_b3:
                ones1_sb = singles.tile([1, S], BF16, tag="ones1")
                nc.vector.memset(ones1_sb, 1.0)
                b3_sb = singles.tile([1, COUT], BF16, tag="b3")
                nc.sync.dma_start(out=b3_sb, in_=b3_d[:, :])

            # ---- prefetch features (cast f32 -> bf16 during SWDGE DMA) ----
            x_sb = []
            for b in range(B_CORE):
                xs = []
                for i in range(CIN // 128):
                    t = xpool.tile([128, HW], BF16, tag="x")
                    nc.gpsimd.dma_start(out=t, in_=feat_d[b, i * 128:(i + 1) * 128, :])
                    xs.append(t)
                x_sb.append(xs)

            for _rep in range(reps):
                # ---- Phase 0: masks -> A (both images) ----
                # A_sb[b]: [80(q), 8*80(k,p)] bf16 ; A[k, p, q] at [q, k*80+p]
                a_sb = []
                for b in range(B_CORE):
                    if 'masks' in skip:
                        at = apool.tile([S, K * S], BF16, tag="a")
                        nc.vector.memset(at[:, 0:1], 0.0)
                        a_sb.append(at)
                        continue
                    lab1 = mpool.tile([128, S2], I32, tag="lab1")
                    nc.sync.dma_start(out=lab1, in_=lab_d[b, 0:128, :])
                    lab2 = mpool.tile([32, S2], I32, tag="lab2")
                    nc.sync.dma_start(out=lab2, in_=lab_d[b, 128:S2, :])
                    m1 = mpool.tile([128, K * S2], BF16, tag="m1")
                    m2 = mpool.tile([32, K * S2], BF16, tag="m2")
                    for k in range(K):
                        nc.vector.tensor_scalar(
                            out=m1[:, k * S2:(k + 1) * S2], in0=lab1,
                            scalar1=float(k), scalar2=None, op0=AluOp.is_equal)
                        nc.vector.tensor_scalar(
                            out=m2[:, k * S2:(k + 1) * S2], in0=lab2,
                            scalar1=float(k), scalar2=None, op0=AluOp.is_equal)
                    # step 1: Tt[j, (k,p)] = sum_i M[i, j] * Rh[i, p]
                    tp = {}
                    tp[(0, 0)] = psA.tile([128, 6 * S], F32, tag="c1", name="tp00")
                    tp[(0, 1)] = psA.tile([128, 2 * S], F32, tag="c1", name="tp01")
                    tp[(1, 0)] = psB.tile([32, 6 * S], F32, tag="c2", name="tp10")
                    tp[(1, 1)] = psB.tile([32, 2 * S], F32, tag="c2", name="tp11")
                    jchunks = [(0, 128), (128, 32)]
                    ichunks = [(0, 128, m1, rh_sb1), (128, 32, m2, rh_sb2)]
                    for jci, (j0, jn) in enumerate(jchunks):
                        for k in range(K):
                            grp = 0 if k < 6 else 1
                            col = (k if k < 6 else k - 6) * S
                            for ici, (i0, in_, mt, rht) in enumerate(ichunks):
                                nc.tensor.matmul(
                                    tp[(jci, grp)][:, col:col + S],
                                    lhsT=mt[:, k * S2 + j0:k * S2 + j0 + jn],
                                    rhs=rht[:, :],
                                    start=(ici == 0), stop=(ici == 1))
                    t1s = tpool.tile([128, K * S], BF16, tag="t1s")
                    t2s = tpool.tile([32, K * S], BF16, tag="t2s")
                    nc.scalar.activation(out=t1s[:, 0:6 * S], in_=tp[(0, 0)], func=ActFn.Copy)
                    nc.scalar.activation(out=t1s[:, 6 * S:8 * S], in_=tp[(0, 1)], func=ActFn.Copy)
                    nc.scalar.activation(out=t2s[:, 0:6 * S], in_=tp[(1, 0)], func=ActFn.Copy)
                    nc.scalar.activation(out=t2s[:, 6 * S:8 * S], in_=tp[(1, 1)], func=ActFn.Copy)
                    # step 2: At[q, (k,p)] = sum_j Rw[j, q] * Tt[j, (k,p)]
                    ap_a = psC.tile([S, 6 * S], F32, tag="f2t")
                    ap_b = psC.tile([S, 2 * S], F32, tag="f2t")
                    for k in range(K):
                        grp_t, col = (ap_a, k * S) if k < 6 else (ap_b, (k - 6) * S)
                        for jci, (j0, jn, ts_, rht) in enumerate(
                                [(0, 128, t1s, rh_sb1), (128, 32, t2s, rh_sb2)]):
                            nc.tensor.matmul(
                                grp_t[:, col:col + S],
                                lhsT=rht[:, :],
                                rhs=ts_[:, k * S:(k + 1) * S],
                                start=(jci == 0), stop=(jci == 1))
                    at = apool.tile([S, K * S], BF16, tag="a")
                    nc.scalar.activation(out=at[:, 0:6 * S], in_=ap_a, func=ActFn.Copy)
                    nc.scalar.activation(out=at[:, 6 * S:8 * S], in_=ap_b, func=ActFn.Copy)
                    a_sb.append(at)

                # ---- Phase 1: conv chain + pooling ----
                for b in range(B_CORE):
                    a_kp = a_sb[b].rearrange("q (k p) -> q k p", k=K)
                    ppool_t = psP.tile([K, COUT + 1], F32, tag="pp", name=f"pp{b}")
                    pslice = ppool_t
                    h2big = [h2pool.tile([128, HW], BF16, tag="h2big", name=f"h2big{b}_{_m}")
                             for _m in range(CIN // 128)]
                    c0 = 0
                    for cn in CHUNKS:
                        # conv1
                        h1_t = []
                        for m in range(CMID // 128):
                            ph = psA.tile([128, 512], F32, tag="c1")
                            for kk in range(CIN // 128):
                                nc.tensor.matmul(
                                    ph[:, 0:cn],
                                    lhsT=w1t_sb[kk][:, m * 128:(m + 1) * 128],
                                    rhs=x_sb[b][kk][:, c0:c0 + cn],
                                    start=(kk == 0), stop=(kk == CIN // 128 - 1))
                            ht = h1pool.tile([128, 512], BF16, tag="h1")
                            nc.scalar.activation(out=ht[:, 0:cn], in_=ph[:, 0:cn],
                                                 func=ActFn.Relu,
                                                 bias=b1_sb[:, m:m + 1])
                            h1_t.append(ht)
                        # conv2 -> H2 written into whole-image buffer
                        if 'conv2' not in skip:
                            for m in range(CIN // 128):
                                ph = psB.tile([128, 512], F32, tag="c2")
                                for kk in range(CMID // 128):
                                    nc.tensor.matmul(
                                        ph[:, 0:cn],
                                        lhsT=w2t_sb[kk][:, m * 128:(m + 1) * 128],
                                        rhs=h1_t[kk][:, 0:cn],
                                        start=(kk == 0), stop=(kk == CMID // 128 - 1))
                                nc.scalar.activation(out=h2big[m][:, c0:c0 + cn],
                                                     in_=ph[:, 0:cn],
                                                     func=ActFn.Relu,
                                                     bias=b2_sb[:, m:m + 1])
                        c0 += cn
                    # conv3 (transposed output) + pooling, per p-row
                    for pg in (range(S) if 'conv3' not in skip else []):
                        pf = psC.tile([S, COUT], F32, tag="f2t")
                        if with_b3:
                            nc.tensor.matmul(pf, lhsT=ones1_sb, rhs=b3_sb,
                                             start=True, stop=False)
                        for kk in range(CIN // 128):
                            nc.tensor.matmul(
                                pf,
                                lhsT=h2big[kk][:, pg * S:(pg + 1) * S],
                                rhs=w3t_sb[kk][:, :],
                                start=(kk == 0 and not with_b3),
                                stop=(kk == CIN // 128 - 1))
                        f2t = f2t_tiles[pg % 3]
                        nc.vector.tensor_scalar_max(f2t[:, 0:COUT], pf, 0.0)
                        # pooled[k, :COUT] += sum_q A[k, pg, q] f2t[q, :]
                        # ones column accumulates the per-class counts
                        nc.tensor.matmul(
                            pslice, lhsT=a_kp[:, :, pg], rhs=f2t,
                            start=(pg == 0), stop=(pg == S - 1),
                            skip_group_check=True)
                    if 'conv3' in skip:
                        nc.tensor.matmul(pslice, lhsT=a_kp[:, :, 0],
                                         rhs=f2t_tiles[0], start=True, stop=True)
                    # finalize image
                    cnt = smpool.tile([K, 1], F32, tag="cnt")
                    nc.vector.tensor_scalar_add(cnt, pslice[:, COUT:COUT + 1], EPS)
                    rec = smpool.tile([K, 1], F32, tag="rec")
                    nc.vector.reciprocal(rec, cnt)
                    po = smpool.tile([K, COUT], F32, tag="po")
                    nc.vector.tensor_scalar(out=po, in0=pslice[:, 0:COUT],
                                            scalar1=rec, scalar2=None, op0=AluOp.mult)
                    pr = smpool.tile([K, 1], I32, tag="pr")
                    nc.vector.tensor_scalar(out=pr, in0=pslice[:, COUT:COUT + 1],
                                            scalar1=0.0, scalar2=None,
                                            op0=AluOp.not_equal)
                    nc.sync.dma_start(out=pooled_d[b, :, :], in_=po)
                    nc.sync.dma_start(out=pres_d[b, :], in_=pr[:, 0])

    nc.compile()
    return nc


_PROGRAM_CACHE = {}


def _get_program(with_b3: bool, reps: int = 1, skip=()):
    key = (with_b3, reps, tuple(skip))
    if key not in _PROGRAM_CACHE:
        _PROGRAM_CACHE[key] = _build_program(with_b3, reps, skip)
    return _PROGRAM_CACHE[key]


def _prep_inputs(feature, fusion_label, w1, b1, w2, b2, w3, b3):
    import ml_dtypes
    bf16 = ml_dtypes.bfloat16
    feature = np.ascontiguousarray(np.asarray(feature, dtype=np.float32))
    label = np.ascontiguousarray(np.asarray(fusion_label).astype(np.int32))
    b3 = np.asarray(b3, dtype=np.float32)
    with_b3 = bool(np.any(b3 != 0))
    w1t = np.ascontiguousarray(np.asarray(w1, np.float32).T).astype(bf16)
    w2t = np.ascontiguousarray(np.asarray(w2, np.float32).T).astype(bf16)
    w3t = np.ascontiguousarray(np.asarray(w3, np.float32).T).astype(bf16)
    b1c = np.ascontiguousarray(np.asarray(b1, np.float32).reshape(CMID // 128, 128).T)
    b2c = np.ascontiguousarray(np.asarray(b2, np.float32).reshape(CIN // 128, 128).T)
    rh = _resize_matrix(S, S2).astype(bf16)
    in_maps = []
    for core in range(N_CORES):
        b0 = core * B_CORE
        m = {"feature": feature[b0:b0 + B_CORE].reshape(B_CORE, CIN, HW),
             "label": label[b0:b0 + B_CORE, 0],
             "w1t": w1t, "w2t": w2t, "w3t": w3t,
             "b1c": b1c, "b2c": b2c, "rh": rh}
        if with_b3:
            m["b3r"] = b3.reshape(1, COUT).astype(bf16)
        in_maps.append(m)
    return in_maps, with_b3


def kernel(feature, fusion_label, w1, b1, w2, b2, w3, b3, _trace=False):
    global LAST_RESULT
    in_maps, with_b3 = _prep_inputs(feature, fusion_label, w1, b1, w2, b2, w3, b3)
    nc = _get_program(with_b3)
    res = bass_utils.run_bass_kernel_spmd(
        nc, in_maps, core_ids=list(range(N_CORES)), trace=_trace)
    LAST_RESULT = res
    pooled = np.concatenate([r["pooled"] for r in res.results], axis=0)
    pres = np.concatenate([r["presence"] for r in res.results], axis=0)
    return pooled.astype(np.float32), pres.astype(np.int32)


def bench(feature, fusion_label, w1, b1, w2, b2, w3, b3, iters=10, reps=1, skip=()):
    """Times device execution with device-resident inputs."""
    import time
    import jax
    from jax.sharding import Mesh, PartitionSpec, NamedSharding
    from jax.experimental.shard_map import shard_map
    from concourse import bass2jax, mybir as _mb

    in_maps, with_b3 = _prep_inputs(feature, fusion_label, w1, b1, w2, b2, w3, b3)
    nc = _get_program(with_b3, reps, skip)

    bass2jax.install_neuronx_cc_hook()
    partition_name = nc.partition_id_tensor.name if nc.partition_id_tensor else None
    in_names, out_names, out_avals, zero_outs = [], [], [], []
    for alloc in nc.m.functions[0].allocations:
        if not isinstance(alloc, _mb.MemoryLocationSet):
            continue
        name = alloc.memorylocations[0].name
        if alloc.kind == "ExternalInput":
            if name != partition_name:
                in_names.append(name)
        elif alloc.kind == "ExternalOutput":
            out_names.append(name)
            shape = tuple(alloc.tensor_shape)
            dtype = _mb.dt.np(alloc.dtype)
            out_avals.append(jax.core.ShapedArray(shape, dtype))
            zero_outs.append(np.zeros(shape, dtype))
    n_params = len(in_names)
    all_in_names = list(in_names) + out_names
    if partition_name is not None:
        all_in_names.append(partition_name)

    def _body(*args):
        operands = list(args)
        if partition_name is not None:
            operands.append(bass2jax.partition_id_tensor())
        outs = bass2jax._bass_exec_p.bind(
            *operands,
            out_avals=tuple(out_avals),
            in_names=tuple(all_in_names),
            out_names=tuple(out_names),
            lowering_input_output_aliases=(),
            sim_require_finite=True, sim_require_nnan=True, nc=nc)
        return tuple(outs)

    devices = jax.devices()[:N_CORES]
    mesh = Mesh(np.asarray(devices), ("core",))
    n_outs = len(out_names)
    donate = tuple(range(n_params, n_params + n_outs))
    sharded = jax.jit(
        shard_map(_body, mesh=mesh,
                  in_specs=(PartitionSpec("core"),) * (n_params + n_outs),
                  out_specs=(PartitionSpec("core"),) * n_outs, check_rep=False),
        donate_argnums=donate, keep_unused=True)

    concat_in = [np.concatenate([np.asarray(in_maps[c][nm]) for c in range(N_CORES)], axis=0)
                 for nm in in_names]
    sh = NamedSharding(mesh, PartitionSpec("core"))
    dev_in = [jax.device_put(x, sh) for x in concat_in]

    def zeros():
        return [jax.device_put(np.zeros((N_CORES * z.shape[0], *z.shape[1:]), z.dtype), sh)
                for z in zero_outs]

    outs = sharded(*dev_in, *zeros())
    jax.block_until_ready(outs)
    times = []
    for _ in range(iters):
        zs = zeros()
        jax.block_until_ready(zs)
        t0 = time.perf_counter()
        outs = sharded(*dev_in, *zs)
        jax.block_until_ready(outs)
        times.append(time.perf_counter() - t0)
    times = np.array(times) * 1e9
    return float(times.min()), float(times.mean())


# revision 21
# speedup vs baseline: 1.1488x; 1.0288x over previous
"""Trainium2 Bass kernel for nn_Pix_Projector2 (segment_reduce).

Computation (per batch image):
  h1 = relu(w1 @ x + b1)          x: [256, 6400]   h1: [512, 6400]
  h2 = relu(w2 @ h1 + b2)                          h2: [256, 6400]
  f2 = relu(w3 @ h2 + b3)                          f2: [128, 6400]
  f3 = bilinear_resize(f2, 80x80 -> 160x160)
  pooled[k, c] = sum_{IJ} onehot(label)[I,J,k] * f3[c,I,J] / (count[k]+eps)
  presence[k]  = count[k] > 0

Key algebraic optimization: the bilinear resize R is linear and separable
(f3 = Rh f2 Rw^T per channel), so instead of upsampling f2 (4x data) we
downsample the one-hot masks:
  A_k = Rh^T M_k Rw   (A: [8, 80, 80], exactly representable in bf16)
  pooled[k, c] = sum_pq A[k, pq] f2[c, pq]
  count[k] = sum_pq A[k, pq]     (rows of R sum to 1 -> counts exact)

conv3 uses H2 chunks as the stationary operand so its output lands
transposed ([pq, c]), which the pooling matmul needs (contraction over pq
must be on the partition axis). Pooling accumulates [8, 129] (128
channels + a ones-column that yields the per-class counts) over 80 p-row
matmuls in a single PSUM accumulation group.

Sharding: batch 16 -> 2 images per core across 8 cores; weights replicated.
"""

import os
import numpy as np

import concourse.bass as bass
import concourse.bacc as bacc
import concourse.tile as tile
import concourse.mybir as mybir
from concourse import bass_utils

F32 = mybir.dt.float32
BF16 = mybir.dt.bfloat16
I32 = mybir.dt.int32

N_CORES = 8
B_TOTAL = 16
B_CORE = B_TOTAL // N_CORES  # 2 images per core
CIN = 256
CMID = 512
COUT = 128
HW = 6400          # 80*80
S = 80             # low-res spatial side
S2 = 160           # label spatial side
K = 8              # classes
EPS = 1e-8
CHUNKS = [512] * 12 + [256]   # pq chunking for conv1/conv2

AluOp = mybir.AluOpType
ActFn = mybir.ActivationFunctionType

LAST_RESULT = None  # BassKernelResults of the most recent run (for test.py)


def _resize_matrix(n_in, n_out):
    """jax.image.resize(method='bilinear', antialias=False) weight matrix
    [n_out, n_in] with half-pixel centers."""
    scale = n_out / n_in
    x = (np.arange(n_out) + 0.5) / scale - 0.5
    p = np.arange(n_in)
    w = np.maximum(0.0, 1.0 - np.abs(x[:, None] - p[None, :]))
    tot = w.sum(1, keepdims=True)
    w = np.where(tot > 0, w / tot, 0.0)
    return w.astype(np.float32)


def _build_program(with_b3: bool, reps: int = 1, skip=()):
    """Builds the per-core bass program. `reps`/`skip` are for timing only."""
    nc = bacc.Bacc("TRN2", target_bir_lowering=False, debug=False,
                   num_devices=N_CORES)

    feat_d = nc.dram_tensor("feature", [B_CORE, CIN, HW], F32, kind="ExternalInput")
    lab_d = nc.dram_tensor("label", [B_CORE, S2, S2], I32, kind="ExternalInput")
    w1t_d = nc.dram_tensor("w1t", [CIN, CMID], BF16, kind="ExternalInput")
    w2t_d = nc.dram_tensor("w2t", [CMID, CIN], BF16, kind="ExternalInput")
    w3t_d = nc.dram_tensor("w3t", [CIN, COUT], BF16, kind="ExternalInput")
    b1_d = nc.dram_tensor("b1c", [128, CMID // 128], F32, kind="ExternalInput")
    b2_d = nc.dram_tensor("b2c", [128, CIN // 128], F32, kind="ExternalInput")
    rh_d = nc.dram_tensor("rh", [S2, S], BF16, kind="ExternalInput")
    if with_b3:
        b3_d = nc.dram_tensor("b3r", [1, COUT], BF16, kind="ExternalInput")
    pooled_d = nc.dram_tensor("pooled", [B_CORE, K, COUT], F32, kind="ExternalOutput")
    pres_d = nc.dram_tensor("presence", [B_CORE, K], I32, kind="ExternalOutput")

    with tile.TileContext(nc) as tc:
        import contextlib
        ctx = contextlib.ExitStack()
        with ctx:
            singles = ctx.enter_context(tc.tile_pool(name="singles", bufs=1))
            xpool = ctx.enter_context(tc.tile_pool(name="x", bufs=2 * B_CORE))
            mpool = ctx.enter_context(tc.tile_pool(name="masks", bufs=2))
            tpool = ctx.enter_context(tc.tile_pool(name="tsb", bufs=2))
            apool = ctx.enter_context(tc.tile_pool(name="asb", bufs=B_CORE))
            h1pool = ctx.enter_context(tc.tile_pool(name="h1", bufs=8))
            h2pool = ctx.enter_context(tc.tile_pool(name="h2big", bufs=2 * B_CORE))
            f2pool = ctx.enter_context(tc.tile_pool(name="f2t", bufs=1))
            smpool = ctx.enter_context(tc.tile_pool(name="small", bufs=4))
            # PSUM: psA 3 + psB 2 + psC 2 + psP 1 = 8 banks
            psA = ctx.enter_context(tc.tile_pool(name="psA", bufs=3, space="PSUM"))
            psB = ctx.enter_context(tc.tile_pool(name="psB", bufs=2, space="PSUM"))
            psC = ctx.enter_context(tc.tile_pool(name="psC", bufs=2, space="PSUM"))
            psP = ctx.enter_context(tc.tile_pool(name="psP", bufs=1, space="PSUM"))

            # ---- weights / constants ----
            w1t_sb = []
            for i in range(CIN // 128):
                t = singles.tile([128, CMID], BF16, tag=f"w1t{i}")
                nc.sync.dma_start(out=t, in_=w1t_d[i * 128:(i + 1) * 128, :])
                w1t_sb.append(t)
            w2t_sb = []
            for i in range(CMID // 128):
                t = singles.tile([128, CIN], BF16, tag=f"w2t{i}")
                nc.sync.dma_start(out=t, in_=w2t_d[i * 128:(i + 1) * 128, :])
                w2t_sb.append(t)
            w3t_sb = []
            for i in range(CIN // 128):
                t = singles.tile([128, COUT], BF16, tag=f"w3t{i}")
                nc.sync.dma_start(out=t, in_=w3t_d[i * 128:(i + 1) * 128, :])
                w3t_sb.append(t)
            b1_sb = singles.tile([128, CMID // 128], F32, tag="b1")
            nc.sync.dma_start(out=b1_sb, in_=b1_d[:, :])
            b2_sb = singles.tile([128, CIN // 128], F32, tag="b2")
            nc.sync.dma_start(out=b2_sb, in_=b2_d[:, :])
            rh_sb1 = singles.tile([128, S], BF16, tag="rh1")
            nc.sync.dma_start(out=rh_sb1, in_=rh_d[0:128, :])
            rh_sb2 = singles.tile([32, S], BF16, tag="rh2")
            nc.sync.dma_start(out=rh_sb2, in_=rh_d[128:S2, :])
            # f2t tiles (4 p-row blocks of [80, 129] each) rotated manually;
            # ones-columns (count accumulators) set once
            f2t_tiles = []
            for i in range(3):
                ft = f2pool.tile([S, 4 * (COUT + 1)], BF16, tag=f"f2_{i}",
                                 name=f"f2t{i}")
                nc.vector.memset(
                    ft.rearrange("q (r u) -> q r u", u=COUT + 1)[:, :, COUT:COUT + 1],
                    1.0)
                f2t_tiles.append(ft)
            if with_b3:
                ones1_sb = singles.tile([1, S], BF16, tag="ones1")
                nc.vector.memset(ones1_sb, 1.0)
                b3_sb = singles.tile([1, COUT], BF16, tag="b3")
                nc.sync.dma_start(out=b3_sb, in_=b3_d[:, :])

            # ---- prefetch features (cast f32 -> bf16 during SWDGE DMA) ----
            x_sb = []
            for b in range(B_CORE):
                xs = []
                for i in range(CIN // 128):
                    t = xpool.tile([128, HW], BF16, tag="x")
                    nc.gpsimd.dma_start(out=t, in_=feat_d[b, i * 128:(i + 1) * 128, :])
                    xs.append(t)
                x_sb.append(xs)

            for _rep in range(reps):
                # ---- Phase 0: masks -> A (both images) ----
                # A_sb[b]: [80(q), 8*80(k,p)] bf16 ; A[k, p, q] at [q, k*80+p]
                a_sb = []
                for b in range(B_CORE):
                    if 'masks' in skip:
                        at = apool.tile([S, K * S], BF16, tag="a")
                        nc.vector.memset(at[:, 0:1], 0.0)
                        a_sb.append(at)
                        continue
                    lab1 = mpool.tile([128, S2], I32, tag="lab1")
                    nc.sync.dma_start(out=lab1, in_=lab_d[b, 0:128, :])
                    lab2 = mpool.tile([32, S2], I32, tag="lab2")
                    nc.sync.dma_start(out=lab2, in_=lab_d[b, 128:S2, :])
                    m1 = mpool.tile([128, K * S2], BF16, tag="m1")
                    m2 = mpool.tile([32, K * S2], BF16, tag="m2")
                    for k in range(K):
                        nc.vector.tensor_scalar(
                            out=m1[:, k * S2:(k + 1) * S2], in0=lab1,
                            scalar1=float(k), scalar2=None, op0=AluOp.is_equal)
                        nc.vector.tensor_scalar(
                            out=m2[:, k * S2:(k + 1) * S2], in0=lab2,
                            scalar1=float(k), scalar2=None, op0=AluOp.is_equal)
                    # step 1: Tt[j, (k,p)] = sum_i M[i, j] * Rh[i, p]
                    tp = {}
                    tp[(0, 0)] = psA.tile([128, 6 * S], F32, tag="c1", name="tp00")
                    tp[(0, 1)] = psA.tile([128, 2 * S], F32, tag="c1", name="tp01")
                    tp[(1, 0)] = psB.tile([32, 6 * S], F32, tag="c2", name="tp10")
                    tp[(1, 1)] = psB.tile([32, 2 * S], F32, tag="c2", name="tp11")
                    jchunks = [(0, 128), (128, 32)]
                    ichunks = [(0, 128, m1, rh_sb1), (128, 32, m2, rh_sb2)]
                    for jci, (j0, jn) in enumerate(jchunks):
                        for k in range(K):
                            grp = 0 if k < 6 else 1
                            col = (k if k < 6 else k - 6) * S
                            for ici, (i0, in_, mt, rht) in enumerate(ichunks):
                                nc.tensor.matmul(
                                    tp[(jci, grp)][:, col:col + S],
                                    lhsT=mt[:, k * S2 + j0:k * S2 + j0 + jn],
                                    rhs=rht[:, :],
                                    start=(ici == 0), stop=(ici == 1))
                    t1s = tpool.tile([128, K * S], BF16, tag="t1s")
                    t2s = tpool.tile([32, K * S], BF16, tag="t2s")
                    nc.scalar.activation(out=t1s[:, 0:6 * S], in_=tp[(0, 0)], func=ActFn.Copy)
                    nc.scalar.activation(out=t1s[:, 6 * S:8 * S], in_=tp[(0, 1)], func=ActFn.Copy)
                    nc.scalar.activation(out=t2s[:, 0:6 * S], in_=tp[(1, 0)], func=ActFn.Copy)
                    nc.scalar.activation(out=t2s[:, 6 * S:8 * S], in_=tp[(1, 1)], func=ActFn.Copy)
                    # step 2: At[q, (k,p)] = sum_j Rw[j, q] * Tt[j, (k,p)]
                    ap_a = psC.tile([S, 6 * S], F32, tag="f2t")
                    ap_b = psC.tile([S, 2 * S], F32, tag="f2t")
                    for k in range(K):
                        grp_t, col = (ap_a, k * S) if k < 6 else (ap_b, (k - 6) * S)
                        for jci, (j0, jn, ts_, rht) in enumerate(
                                [(0, 128, t1s, rh_sb1), (128, 32, t2s, rh_sb2)]):
                            nc.tensor.matmul(
                                grp_t[:, col:col + S],
                                lhsT=rht[:, :],
                                rhs=ts_[:, k * S:(k + 1) * S],
                                start=(jci == 0), stop=(jci == 1))
                    at = apool.tile([S, K * S], BF16, tag="a")
                    nc.scalar.activation(out=at[:, 0:6 * S], in_=ap_a, func=ActFn.Copy)
                    nc.scalar.activation(out=at[:, 6 * S:8 * S], in_=ap_b, func=ActFn.Copy)
                    a_sb.append(at)

                # ---- Phase 1: conv chain + pooling ----
                for b in range(B_CORE):
                    a_kp = a_sb[b].rearrange("q (k p) -> q k p", k=K)
                    ppool_t = psP.tile([K, COUT + 1], F32, tag="pp", name=f"pp{b}")
                    pslice = ppool_t
                    h2big = [h2pool.tile([128, HW], BF16, tag="h2big", name=f"h2big{b}_{_m}")
                             for _m in range(CIN // 128)]

                    def conv3_pool_group(g):
                        # 4 p-rows per PSUM tile; the first matmul's start=True
                        # clears the whole bank, later sub-blocks are fresh
                        # writes (has_written clear -> overwrite) then
                        # accumulate, so one group suffices.
                        pf = psC.tile([S, 4 * COUT], F32, tag="f2t",
                                      name=f"pf{b}_{g}")
                        if with_b3:
                            nc.tensor.matmul(pf[:, 0:COUT], lhsT=ones1_sb,
                                             rhs=b3_sb, start=True, stop=False,
                                             skip_group_check=True)
                        for r in range(4):
                            pg = 4 * g + r
                            if with_b3 and r > 0:
                                nc.tensor.matmul(
                                    pf[:, r * COUT:(r + 1) * COUT],
                                    lhsT=ones1_sb, rhs=b3_sb,
                                    start=False, stop=False, skip_group_check=True)
                            for kk in range(CIN // 128):
                                nc.tensor.matmul(
                                    pf[:, r * COUT:(r + 1) * COUT],
                                    lhsT=h2big[kk][:, pg * S:(pg + 1) * S],
                                    rhs=w3t_sb[kk][:, :],
                                    start=(r == 0 and kk == 0 and not with_b3),
                                    stop=(r == 3 and kk == CIN // 128 - 1),
                                    skip_group_check=True)
                        ft = f2t_tiles[g % 3]
                        # relu all 4 rows in one strided DVE op
                        nc.vector.tensor_scalar_max(
                            ft.rearrange("q (r u) -> q r u", u=COUT + 1)[:, :, 0:COUT],
                            pf.rearrange("q (r u) -> q r u", u=COUT), 0.0)
                        for r in range(4):
                            pg = 4 * g + r
                            # pooled[k, :COUT] += sum_q A[k, pg, q] f2[q, :]
                            # ones column accumulates the per-class counts
                            nc.tensor.matmul(
                                pslice,
                                lhsT=a_kp[:, :, pg],
                                rhs=ft[:, r * (COUT + 1):(r + 1) * (COUT + 1)],
                                start=(pg == 0), stop=(pg == S - 1),
                                skip_group_check=True)

                    c0 = 0
                    g_done = 0
                    for cn in CHUNKS:
                        # conv1
                        h1_t = []
                        for m in range(CMID // 128):
                            ph = psA.tile([128, 512], F32, tag="c1")
                            for kk in range(CIN // 128):
                                nc.tensor.matmul(
                                    ph[:, 0:cn],
                                    lhsT=w1t_sb[kk][:, m * 128:(m + 1) * 128],
                                    rhs=x_sb[b][kk][:, c0:c0 + cn],
                                    start=(kk == 0), stop=(kk == CIN // 128 - 1))
                            ht = h1pool.tile([128, 512], BF16, tag="h1")
                            nc.scalar.activation(out=ht[:, 0:cn], in_=ph[:, 0:cn],
                                                 func=ActFn.Relu,
                                                 bias=b1_sb[:, m:m + 1])
                            h1_t.append(ht)
                        # conv2 -> H2 whole-image buffer; bias+relu fused on DVE
                        if 'conv2' not in skip:
                            for m in range(CIN // 128):
                                ph = psB.tile([128, 512], F32, tag="c2")
                                for kk in range(CMID // 128):
                                    nc.tensor.matmul(
                                        ph[:, 0:cn],
                                        lhsT=w2t_sb[kk][:, m * 128:(m + 1) * 128],
                                        rhs=h1_t[kk][:, 0:cn],
                                        start=(kk == 0), stop=(kk == CMID // 128 - 1))
                                nc.vector.tensor_scalar(
                                    out=h2big[m][:, c0:c0 + cn], in0=ph[:, 0:cn],
                                    scalar1=b2_sb[:, m:m + 1], scalar2=0.0,
                                    op0=AluOp.add, op1=AluOp.max)
                        c0 += cn
                        # conv3+pool for p-row groups whose H2 data is complete
                        if 'conv3' not in skip:
                            g_avail = c0 // (4 * S)
                            for g in range(g_done, g_avail):
                                conv3_pool_group(g)
                            g_done = g_avail
                    if 'conv3' in skip:
                        nc.tensor.matmul(pslice, lhsT=a_kp[:, :, 0],
                                         rhs=f2t_tiles[0][:, 0:COUT + 1],
                                         start=True, stop=True)
                    # finalize image
                    cnt = smpool.tile([K, 1], F32, tag="cnt")
                    nc.vector.tensor_scalar_add(cnt, pslice[:, COUT:COUT + 1], EPS)
                    rec = smpool.tile([K, 1], F32, tag="rec")
                    nc.vector.reciprocal(rec, cnt)
                    po = smpool.tile([K, COUT], F32, tag="po")
                    nc.vector.tensor_scalar(out=po, in0=pslice[:, 0:COUT],
                                            scalar1=rec, scalar2=None, op0=AluOp.mult)
                    pr = smpool.tile([K, 1], I32, tag="pr")
                    nc.vector.tensor_scalar(out=pr, in0=pslice[:, COUT:COUT + 1],
                                            scalar1=0.0, scalar2=None,
                                            op0=AluOp.not_equal)
                    nc.sync.dma_start(out=pooled_d[b, :, :], in_=po)
                    nc.sync.dma_start(out=pres_d[b, :], in_=pr[:, 0])

    nc.compile()
    return nc


_PROGRAM_CACHE = {}


def _get_program(with_b3: bool, reps: int = 1, skip=()):
    key = (with_b3, reps, tuple(skip))
    if key not in _PROGRAM_CACHE:
        _PROGRAM_CACHE[key] = _build_program(with_b3, reps, skip)
    return _PROGRAM_CACHE[key]


def _prep_inputs(feature, fusion_label, w1, b1, w2, b2, w3, b3):
    import ml_dtypes
    bf16 = ml_dtypes.bfloat16
    feature = np.ascontiguousarray(np.asarray(feature, dtype=np.float32))
    label = np.ascontiguousarray(np.asarray(fusion_label).astype(np.int32))
    b3 = np.asarray(b3, dtype=np.float32)
    with_b3 = bool(np.any(b3 != 0))
    w1t = np.ascontiguousarray(np.asarray(w1, np.float32).T).astype(bf16)
    w2t = np.ascontiguousarray(np.asarray(w2, np.float32).T).astype(bf16)
    w3t = np.ascontiguousarray(np.asarray(w3, np.float32).T).astype(bf16)
    b1c = np.ascontiguousarray(np.asarray(b1, np.float32).reshape(CMID // 128, 128).T)
    b2c = np.ascontiguousarray(np.asarray(b2, np.float32).reshape(CIN // 128, 128).T)
    rh = _resize_matrix(S, S2).astype(bf16)
    in_maps = []
    for core in range(N_CORES):
        b0 = core * B_CORE
        m = {"feature": feature[b0:b0 + B_CORE].reshape(B_CORE, CIN, HW),
             "label": label[b0:b0 + B_CORE, 0],
             "w1t": w1t, "w2t": w2t, "w3t": w3t,
             "b1c": b1c, "b2c": b2c, "rh": rh}
        if with_b3:
            m["b3r"] = b3.reshape(1, COUT).astype(bf16)
        in_maps.append(m)
    return in_maps, with_b3


def kernel(feature, fusion_label, w1, b1, w2, b2, w3, b3, _trace=False):
    global LAST_RESULT
    in_maps, with_b3 = _prep_inputs(feature, fusion_label, w1, b1, w2, b2, w3, b3)
    nc = _get_program(with_b3)
    res = bass_utils.run_bass_kernel_spmd(
        nc, in_maps, core_ids=list(range(N_CORES)), trace=_trace)
    LAST_RESULT = res
    pooled = np.concatenate([r["pooled"] for r in res.results], axis=0)
    pres = np.concatenate([r["presence"] for r in res.results], axis=0)
    return pooled.astype(np.float32), pres.astype(np.int32)


def bench(feature, fusion_label, w1, b1, w2, b2, w3, b3, iters=10, reps=1, skip=()):
    """Times device execution with device-resident inputs."""
    import time
    import jax
    from jax.sharding import Mesh, PartitionSpec, NamedSharding
    from jax.experimental.shard_map import shard_map
    from concourse import bass2jax, mybir as _mb

    in_maps, with_b3 = _prep_inputs(feature, fusion_label, w1, b1, w2, b2, w3, b3)
    nc = _get_program(with_b3, reps, skip)

    bass2jax.install_neuronx_cc_hook()
    partition_name = nc.partition_id_tensor.name if nc.partition_id_tensor else None
    in_names, out_names, out_avals, zero_outs = [], [], [], []
    for alloc in nc.m.functions[0].allocations:
        if not isinstance(alloc, _mb.MemoryLocationSet):
            continue
        name = alloc.memorylocations[0].name
        if alloc.kind == "ExternalInput":
            if name != partition_name:
                in_names.append(name)
        elif alloc.kind == "ExternalOutput":
            out_names.append(name)
            shape = tuple(alloc.tensor_shape)
            dtype = _mb.dt.np(alloc.dtype)
            out_avals.append(jax.core.ShapedArray(shape, dtype))
            zero_outs.append(np.zeros(shape, dtype))
    n_params = len(in_names)
    all_in_names = list(in_names) + out_names
    if partition_name is not None:
        all_in_names.append(partition_name)

    def _body(*args):
        operands = list(args)
        if partition_name is not None:
            operands.append(bass2jax.partition_id_tensor())
        outs = bass2jax._bass_exec_p.bind(
            *operands,
            out_avals=tuple(out_avals),
            in_names=tuple(all_in_names),
            out_names=tuple(out_names),
            lowering_input_output_aliases=(),
            sim_require_finite=True, sim_require_nnan=True, nc=nc)
        return tuple(outs)

    devices = jax.devices()[:N_CORES]
    mesh = Mesh(np.asarray(devices), ("core",))
    n_outs = len(out_names)
    donate = tuple(range(n_params, n_params + n_outs))
    sharded = jax.jit(
        shard_map(_body, mesh=mesh,
                  in_specs=(PartitionSpec("core"),) * (n_params + n_outs),
                  out_specs=(PartitionSpec("core"),) * n_outs, check_rep=False),
        donate_argnums=donate, keep_unused=True)

    concat_in = [np.concatenate([np.asarray(in_maps[c][nm]) for c in range(N_CORES)], axis=0)
                 for nm in in_names]
    sh = NamedSharding(mesh, PartitionSpec("core"))
    dev_in = [jax.device_put(x, sh) for x in concat_in]

    def zeros():
        return [jax.device_put(np.zeros((N_CORES * z.shape[0], *z.shape[1:]), z.dtype), sh)
                for z in zero_outs]

    outs = sharded(*dev_in, *zeros())
    jax.block_until_ready(outs)
    times = []
    for _ in range(iters):
        zs = zeros()
        jax.block_until_ready(zs)
        t0 = time.perf_counter()
        outs = sharded(*dev_in, *zs)
        jax.block_until_ready(outs)
        times.append(time.perf_counter() - t0)
    times = np.array(times) * 1e9
    return float(times.min()), float(times.mean())
